# revision 34
# baseline (speedup 1.0000x reference)
"""ConcatCritic pair-grid MLP, v18: host-pretransposed fp16 layouts.

The host unshard/shard step prepares data layouts (pure data movement +
fp16 casts): xsT = x-slab.T, yT/W1T/W2T pre-transposed and permuted, w3
row permuted.  The device does zero layout transposes — its PE stream is
pure matmuls (warmup -> mm1 -> pair-grid loop), so the HAM clock gate
stays at 2.4 GHz throughout and the main loop starts warm.

Index convention (from contiguous [512,n] host rows -> 128 partitions):
  h = 4c + hb  (W1/b1 rows, A4 partitions, w2T rows)
  j = 4c + jc  (y rows, A4/hy columns, pz/acc partitions)
  k = 4p + r   (W2 rows, pz columns, w3/b2 columns)
Host unshard: out_d[c, jc, i] -> score[i, 4c + jc].

Per-core (64 rows of x, everything else replicated):
  setup:  mm1 -> hxbT[h, i](+b1) fp32, hyT[hb][h, j] fp16
  per i:  ACT  A4[:, hb, :] = relu(hyT[hb] + hxbT[:, hb*64+i])   (4 ops, fp16)
          PE   pz[jc][j, k] += A4[:, hb, jc*128:+128].T @ w2T[hb]  (16 matmuls)
          DVE  acc[jc][:, i] = sum_k relu(pz[jc]) * w3bc          (4 fused ops)

The W3 reduction costs zero PE time (fused into the DVE relu). b2 is
zero in this model family; a fallback build adds an exact K=1 matmul
(ones.T @ b2perm) into each psum accumulation when b2 != 0.
"""

import os

import numpy as np

import concourse.bass as bass
import concourse.bacc as bacc
import concourse.mybir as mybir
from concourse import tile
from concourse.bass_utils import run_bass_kernel_spmd

B = 512
D = 128
H = 512
NCORES = 8
BI = B // NCORES  # 64 rows of x per core
HB = H // 128     # 4 h-blocks
JC = B // 128     # 4 j-chunks
FP = mybir.dt.float32
F16 = mybir.dt.float16

Relu = mybir.ActivationFunctionType.Relu
Identity = mybir.ActivationFunctionType.Identity
Add = mybir.AluOpType.add
Max = mybir.AluOpType.max
Mult = mybir.AluOpType.mult
Bypass = mybir.AluOpType.bypass


def build_v18(b2_nonzero: bool = False) -> bass.Bass:
    nc = bacc.Bacc(
        "TRN2",
        target_bir_lowering=False,
        debug=False,
        enable_asserts=False,
    )

    xsT_d = nc.dram_tensor("xsT", [D, BI], F16, kind="ExternalInput")
    yT_d = nc.dram_tensor("yT", [D, B], F16, kind="ExternalInput")
    w1T_d = nc.dram_tensor("w1T", [D, HB, 2, 128], F16, kind="ExternalInput")
    b1_d = nc.dram_tensor("b1", [H], FP, kind="ExternalInput")
    w2T_d = nc.dram_tensor("w2T", [128, HB, H], F16, kind="ExternalInput")
    b2_d = nc.dram_tensor("b2", [H], FP, kind="ExternalInput")
    w3p_d = nc.dram_tensor("w3p", [1, H], FP, kind="ExternalInput")
    b3c_d = nc.dram_tensor("b3c", [1, 1], FP, kind="ExternalInput")
    # out_d[c, jc, i] = score[i, 4c + jc]; host reshapes at unshard.
    out_d = nc.dram_tensor("out", [128, JC, BI], FP, kind="ExternalOutput")

    with tile.TileContext(nc) as tc:
        with (
            tc.tile_pool(name="consts", bufs=1) as consts,
            tc.tile_pool(name="persist", bufs=1) as persist,
            tc.tile_pool(name="load", bufs=1) as load,
            tc.tile_pool(name="work", bufs=3) as work,
            tc.tile_pool(name="ps", bufs=8, space="PSUM") as ps,
        ):
            # HAM warmup: N=512 matmuls on memset garbage while DMAs land —
            # keeps the PE clock gate at 2.4 GHz into mm1 and the main loop.
            warm_src = consts.tile([128, B], F16, name="warm_src")
            nc.gpsimd.memset(warm_src, 0.0)
            warm_ps = ps.tile([128, B], FP, tag="misc", bufs=1, name="warm_ps")
            for _ in range(11):
                nc.tensor.matmul(
                    warm_ps, warm_src[:, :128], warm_src, start=True, stop=True
                )

            # ------- input DMAs: contiguous fp16 pre-transposed loads -------
            xsT = load.tile([D, BI], F16, name="xsT")
            nc.sync.dma_start(xsT, xsT_d[:, :])
            w1T_sb = load.tile([D, HB, 2, 128], F16, name="w1T_sb")
            nc.sync.dma_start(w1T_sb, w1T_d[:, :, :, :])
            yT = load.tile([D, B], F16, name="yT")
            nc.sync.dma_start(yT, yT_d[:, :])
            # w2T issued LAST on scalar (its descgen delay deprioritizes
            # the big 512KB transfer so y/w1 land first; w2T is only needed
            # when the pair-grid loop starts)
            b1c = consts.tile([128, HB], FP, name="b1c")
            nc.scalar.dma_start(b1c, b1_d[:].rearrange("(p r) -> p r", p=128))
            # host-permuted w3 row + b3, broadcast on idle GpSimd
            w3prow = consts.tile([1, H], FP, name="w3prow")
            nc.scalar.dma_start(w3prow, w3p_d[:, :])
            w2T_sb = load.tile([128, HB, H], F16, name="w2T_sb")
            nc.scalar.dma_start(w2T_sb, w2T_d[:, :, :])
            b3c = consts.tile([1, 1], FP, name="b3c")
            nc.scalar.dma_start(b3c, b3c_d[:, :])
            w3bc = consts.tile([128, B], FP, name="w3bc")
            nc.gpsimd.partition_broadcast(w3bc[:, :], w3prow[:, :])
            b3bc = consts.tile([128, 1], FP, name="b3bc")
            nc.gpsimd.partition_broadcast(b3bc[:, :], b3c[:, :])
            if b2_nonzero:
                b2row = consts.tile([1, H], F16, name="b2row")
                b2row32 = consts.tile([1, H], FP, name="b2row32")
                nc.scalar.dma_start(b2row32, b2_d[None, :])
                b2p32 = consts.tile([1, H], FP, name="b2p32")
                for r in range(4):
                    nc.vector.tensor_copy(
                        b2p32[:, r * 128 : (r + 1) * 128], b2row32[:, r::4]
                    )
                nc.vector.tensor_copy(b2row, b2p32)
                ones_st = consts.tile([1, 128], F16, name="ones_st")
                nc.vector.memset(ones_st, 1.0)

            # ---------------- mm1 (no transposes needed) ----------------
            # hxbT[c, hb*BI + i] = hx[i, 4c+hb] + b1[4c+hb]   (fp32)
            hxbT = persist.tile([128, HB * BI], FP, name="hxbT")
            hyT = [persist.tile([128, B], F16, name=f"hyT{hb}") for hb in range(HB)]
            hy_ps_l = []
            for hb in range(HB):
                hx_ps = ps.tile([128, BI], FP, tag="tbank", bufs=2, name=f"hx_ps{hb}")
                nc.tensor.matmul(
                    hx_ps, w1T_sb[:, hb, 0, :], xsT, start=True, stop=True
                )
                if hb % 2 == 0:
                    nc.vector.tensor_scalar(
                        hxbT[:, hb * BI : (hb + 1) * BI],
                        hx_ps,
                        b1c[:, hb : hb + 1],
                        0.0,
                        Add,
                        Bypass,
                    )
                else:
                    nc.scalar.activation(
                        hxbT[:, hb * BI : (hb + 1) * BI],
                        hx_ps,
                        Identity,
                        bias=b1c[:, hb : hb + 1],
                    )
                hy_ps = ps.tile([128, B], FP, tag="pz", bufs=4, name=f"hy_ps{hb}")
                nc.tensor.matmul(
                    hy_ps, w1T_sb[:, hb, 1, :], yT, start=True, stop=True
                )
                nc.vector.tensor_copy(
                    hyT[hb][:, : B // 2], hy_ps[:, : B // 2]
                )
                nc.scalar.activation(
                    hyT[hb][:, B // 2 :], hy_ps[:, B // 2 :], Identity
                )
                hy_ps_l.append(hy_ps)

            # accumulator staging: acc[jc][c, i] = score[i, 4c + jc]
            acc = [persist.tile([128, BI], FP, name=f"acc{jc}") for jc in range(JC)]
            dummy = persist.tile([128, B], F16, name="stt_dummy")

            # ---------------- main loop ----------------
            def gen_A(i, A4):
                for hb in range(HB):
                    bias = hxbT[:, hb * BI + i : hb * BI + i + 1]
                    if i == 0:
                        # read hy straight from PSUM: skips the hyT-copy dep
                        if hb < 2:
                            nc.scalar.activation(
                                A4[:, hb, :], hy_ps_l[hb], Relu, bias=bias
                            )
                        else:
                            nc.vector.tensor_scalar(
                                A4[:, hb, :], hy_ps_l[hb], bias, 0.0, Add, Max
                            )
                    else:
                        nc.scalar.activation(A4[:, hb, :], hyT[hb], Relu, bias=bias)

            A_bufs = [
                work.tile([128, HB, B], F16, tag="A4", bufs=3, name=f"A4_{p}")
                for p in range(3)
            ]

            gen_A(0, A_bufs[0])
            for i in range(BI):
                A4 = A_bufs[i % 3]
                if i + 1 < BI:
                    gen_A(i + 1, A_bufs[(i + 1) % 3])
                for jc in range(JC):
                    pz = ps.tile(
                        [128, B], FP, tag="pz", bufs=4, name=f"pz{i}_{jc}"
                    )
                    if b2_nonzero:
                        nc.tensor.matmul(pz, ones_st, b2row, start=True, stop=False)
                    for hb in range(HB):
                        nc.tensor.matmul(
                            pz,
                            A4[:, hb, jc * 128 : (jc + 1) * 128],
                            w2T_sb[:, hb, :],
                            start=(hb == 0 and not b2_nonzero),
                            stop=(hb == HB - 1),
                        )
                    # acc[jc][:, i] = sum_k relu(pz) * w3
                    nc.vector.scalar_tensor_tensor(
                        dummy,
                        pz,
                        0.0,
                        w3bc,
                        Max,
                        Mult,
                        accum_out=acc[jc][:, i : i + 1],
                    )

            # ---------------- tail: +b3, DMA out (permuted layout) ----------
            out_q = [nc.sync, nc.scalar, nc.sync, nc.scalar]
            for jc in range(JC):
                outj = persist.tile([128, BI], FP, name=f"outj{jc}")
                nc.vector.tensor_scalar(outj, acc[jc], b3bc, 0.0, Add, Bypass)
                out_q[jc].dma_start(out_d[:, jc, :], outj)

    nc.compile()
    return nc


_BUILT: dict[str, bass.Bass] = {}


def _get_nc(key: str) -> bass.Bass:
    if key not in _BUILT:
        _BUILT[key] = build_v18(b2_nonzero=(key == "b2"))
    return _BUILT[key]


def run(inputs: dict, variant: str | None = None, trace: bool = False):
    x16 = np.asarray(inputs["x"], dtype=np.float32).astype(np.float16)
    y16 = np.asarray(inputs["y"], dtype=np.float32).astype(np.float16)
    W1 = np.asarray(inputs["W1"], dtype=np.float32).astype(np.float16)
    W2 = np.asarray(inputs["W2"], dtype=np.float32).astype(np.float16)
    b1 = np.ascontiguousarray(np.asarray(inputs["b1"], dtype=np.float32))
    b2 = np.ascontiguousarray(np.asarray(inputs["b2"], dtype=np.float32))
    W3 = np.asarray(inputs["W3"], dtype=np.float32)
    b3 = np.asarray(inputs["b3"], dtype=np.float32)

    # yT[d, r*128 + c] = y[4c + r, d]
    yT = np.ascontiguousarray(
        y16.T.reshape(D, 128, 4).transpose(0, 2, 1).reshape(D, B)
    )
    # w1T[d, hb, 0, c] = W1[4c+hb, d]; w1T[d, hb, 1, c] = W1[4c+hb, D+d]
    W1v = W1.reshape(128, 4, 2 * D)
    w1x = W1v[:, :, :D].transpose(2, 1, 0)  # [d, hb, c]
    w1y = W1v[:, :, D:].transpose(2, 1, 0)  # [d, hb, c]
    w1T = np.ascontiguousarray(np.stack((w1x, w1y), axis=2))  # [d, hb, 2, c]
    # w2T[c, hb, r*128 + p] = W2[4p + r, 4c + hb]
    w2T = np.ascontiguousarray(
        W2.reshape(128, 4, 128, 4).transpose(2, 3, 1, 0).reshape(128, HB, H)
    )
    # permuted w3 row: w3p[r*128 + c] = W3[0, 4c + r]
    w3p = np.ascontiguousarray(
        W3[0].reshape(128, 4).T.reshape(1, -1).astype(np.float32)
    )
    b3c = np.ascontiguousarray(b3.reshape(1, 1).astype(np.float32))

    nc = _get_nc("b2" if np.any(b2) else "z")
    in_maps = []
    for c in range(NCORES):
        xsT = np.ascontiguousarray(x16[c * BI : (c + 1) * BI].T)  # [D, BI]
        in_maps.append(
            {
                "xsT": xsT,
                "yT": yT,
                "w1T": w1T,
                "b1": b1,
                "w2T": w2T,
                "b2": b2,
                "w3p": w3p,
                "b3c": b3c,
            }
        )
    res = run_bass_kernel_spmd(nc, in_maps, core_ids=list(range(NCORES)), trace=trace)
    # r["out"][c, jc, i] = score[i, 4c + jc] -> transpose to [i, c, jc] and
    # flatten: column index c*4 + jc = j.
    out = np.concatenate(
        [r["out"].transpose(2, 0, 1).reshape(BI, B) for r in res.results], axis=0
    )
    return np.ascontiguousarray(out), res


def kernel(**inputs) -> np.ndarray:
    out, _ = run(inputs)
    return out


# revision 35
# speedup vs baseline: 1.0109x; 1.0109x over previous
"""ConcatCritic pair-grid MLP, v18: host-pretransposed fp16 layouts.

The host unshard/shard step prepares data layouts (pure data movement +
fp16 casts): xsT = x-slab.T, yT/W1T/W2T pre-transposed and permuted, w3
row permuted.  The device does zero layout transposes — its PE stream is
pure matmuls (warmup -> mm1 -> pair-grid loop), so the HAM clock gate
stays at 2.4 GHz throughout and the main loop starts warm.

Index convention (from contiguous [512,n] host rows -> 128 partitions):
  h = 4c + hb  (W1/b1 rows, A4 partitions, w2T rows)
  j = 4c + jc  (y rows, A4/hy columns, pz/acc partitions)
  k = 4p + r   (W2 rows, pz columns, w3/b2 columns)
Host unshard: out_d[c, jc, i] -> score[i, 4c + jc].

Per-core (64 rows of x, everything else replicated):
  setup:  mm1 -> hxbT[h, i](+b1) fp32, hyT[hb][h, j] fp16
  per i:  ACT  A4[:, hb, :] = relu(hyT[hb] + hxbT[:, hb*64+i])   (4 ops, fp16)
          PE   pz[jc][j, k] += A4[:, hb, jc*128:+128].T @ w2T[hb]  (16 matmuls)
          DVE  acc[jc][:, i] = sum_k relu(pz[jc]) * w3bc          (4 fused ops)

The W3 reduction costs zero PE time (fused into the DVE relu). b2 is
zero in this model family; a fallback build adds an exact K=1 matmul
(ones.T @ b2perm) into each psum accumulation when b2 != 0.
"""

import os

import numpy as np

import concourse.bass as bass
import concourse.bacc as bacc
import concourse.mybir as mybir
from concourse import tile
from concourse.bass_utils import run_bass_kernel_spmd

B = 512
D = 128
H = 512
NCORES = 8
BI = B // NCORES  # 64 rows of x per core
HB = H // 128     # 4 h-blocks
JC = B // 128     # 4 j-chunks
FP = mybir.dt.float32
F16 = mybir.dt.float16

Relu = mybir.ActivationFunctionType.Relu
Identity = mybir.ActivationFunctionType.Identity
Add = mybir.AluOpType.add
Max = mybir.AluOpType.max
Mult = mybir.AluOpType.mult
Bypass = mybir.AluOpType.bypass


def build_v18(b2_nonzero: bool = False) -> bass.Bass:
    nc = bacc.Bacc(
        "TRN2",
        target_bir_lowering=False,
        debug=False,
        enable_asserts=False,
    )

    xsT_d = nc.dram_tensor("xsT", [D, BI], F16, kind="ExternalInput")
    yT_d = nc.dram_tensor("yT", [D, B], F16, kind="ExternalInput")
    w1T_d = nc.dram_tensor("w1T", [D, HB, 2, 128], F16, kind="ExternalInput")
    b1_d = nc.dram_tensor("b1", [H], FP, kind="ExternalInput")
    w2T_d = nc.dram_tensor("w2T", [128, HB, H], F16, kind="ExternalInput")
    b2_d = nc.dram_tensor("b2", [H], FP, kind="ExternalInput")
    w3p_d = nc.dram_tensor("w3p", [1, H], FP, kind="ExternalInput")
    b3c_d = nc.dram_tensor("b3c", [1, 1], FP, kind="ExternalInput")
    # out_d[c, jc, i] = score[i, 4c + jc]; host reshapes at unshard.
    out_d = nc.dram_tensor("out", [128, JC, BI], FP, kind="ExternalOutput")

    with tile.TileContext(nc) as tc:
        with (
            tc.tile_pool(name="consts", bufs=1) as consts,
            tc.tile_pool(name="persist", bufs=1) as persist,
            tc.tile_pool(name="load", bufs=1) as load,
            tc.tile_pool(name="work", bufs=3) as work,
            tc.tile_pool(name="ps", bufs=8, space="PSUM") as ps,
        ):
            # HAM warmup: N=512 matmuls on memset garbage while DMAs land —
            # keeps the PE clock gate at 2.4 GHz into mm1 and the main loop.
            warm_src = consts.tile([128, B], F16, name="warm_src")
            nc.gpsimd.memset(warm_src, 0.0)
            warm_ps = ps.tile([128, B], FP, tag="misc", bufs=1, name="warm_ps")
            for _ in range(11):
                nc.tensor.matmul(
                    warm_ps, warm_src[:, :128], warm_src, start=True, stop=True
                )

            # ------- input DMAs: contiguous fp16 pre-transposed loads -------
            xsT = load.tile([D, BI], F16, name="xsT")
            nc.sync.dma_start(xsT, xsT_d[:, :])
            w1T_sb = load.tile([D, HB, 2, 128], F16, name="w1T_sb")
            nc.sync.dma_start(w1T_sb, w1T_d[:, :, :, :])
            yT = load.tile([D, B], F16, name="yT")
            nc.sync.dma_start(yT, yT_d[:, :])
            # w2T issued LAST on scalar (its descgen delay deprioritizes
            # the big 512KB transfer so y/w1 land first; w2T is only needed
            # when the pair-grid loop starts)
            b1c = consts.tile([128, HB], FP, name="b1c")
            nc.scalar.dma_start(b1c, b1_d[:].rearrange("(p r) -> p r", p=128))
            # host-permuted w3 row + b3, broadcast on idle GpSimd
            w3prow = consts.tile([1, H], FP, name="w3prow")
            nc.scalar.dma_start(w3prow, w3p_d[:, :])
            b3c = consts.tile([1, 1], FP, name="b3c")
            nc.scalar.dma_start(b3c, b3c_d[:, :])
            w2T_sb = load.tile([128, HB, H], F16, name="w2T_sb")
            nc.scalar.dma_start(w2T_sb, w2T_d[:, :, :])
            w3bc = consts.tile([128, B], FP, name="w3bc")
            nc.gpsimd.partition_broadcast(w3bc[:, :], w3prow[:, :])
            b3bc = consts.tile([128, 1], FP, name="b3bc")
            nc.gpsimd.partition_broadcast(b3bc[:, :], b3c[:, :])
            if b2_nonzero:
                b2row = consts.tile([1, H], F16, name="b2row")
                b2row32 = consts.tile([1, H], FP, name="b2row32")
                nc.scalar.dma_start(b2row32, b2_d[None, :])
                b2p32 = consts.tile([1, H], FP, name="b2p32")
                for r in range(4):
                    nc.vector.tensor_copy(
                        b2p32[:, r * 128 : (r + 1) * 128], b2row32[:, r::4]
                    )
                nc.vector.tensor_copy(b2row, b2p32)
                ones_st = consts.tile([1, 128], F16, name="ones_st")
                nc.vector.memset(ones_st, 1.0)

            # ---------------- mm1 (no transposes needed) ----------------
            # hxbT[c, hb*BI + i] = hx[i, 4c+hb] + b1[4c+hb]   (fp32)
            hxbT = persist.tile([128, HB * BI], FP, name="hxbT")
            hyT = [persist.tile([128, B], F16, name=f"hyT{hb}") for hb in range(HB)]
            hy_ps_l = []
            for hb in range(HB):
                hx_ps = ps.tile([128, BI], FP, tag="tbank", bufs=2, name=f"hx_ps{hb}")
                nc.tensor.matmul(
                    hx_ps, w1T_sb[:, hb, 0, :], xsT, start=True, stop=True
                )
                if hb % 2 == 0:
                    nc.vector.tensor_scalar(
                        hxbT[:, hb * BI : (hb + 1) * BI],
                        hx_ps,
                        b1c[:, hb : hb + 1],
                        0.0,
                        Add,
                        Bypass,
                    )
                else:
                    nc.scalar.activation(
                        hxbT[:, hb * BI : (hb + 1) * BI],
                        hx_ps,
                        Identity,
                        bias=b1c[:, hb : hb + 1],
                    )
                hy_ps = ps.tile([128, B], FP, tag="pz", bufs=4, name=f"hy_ps{hb}")
                nc.tensor.matmul(
                    hy_ps, w1T_sb[:, hb, 1, :], yT, start=True, stop=True
                )
                nc.vector.tensor_copy(
                    hyT[hb][:, : B // 2], hy_ps[:, : B // 2]
                )
                nc.scalar.activation(
                    hyT[hb][:, B // 2 :], hy_ps[:, B // 2 :], Identity
                )
                hy_ps_l.append(hy_ps)

            # accumulator staging: acc[jc][c, i] = score[i, 4c + jc]
            acc = [persist.tile([128, BI], FP, name=f"acc{jc}") for jc in range(JC)]
            dummy = persist.tile([128, B], F16, name="stt_dummy")

            # ---------------- main loop ----------------
            def gen_A(i, A4):
                for hb in range(HB):
                    bias = hxbT[:, hb * BI + i : hb * BI + i + 1]
                    if i == 0:
                        nc.vector.tensor_scalar(
                            A4[:, hb, :], hyT[hb], bias, 0.0, Add, Max
                        )
                    else:
                        nc.scalar.activation(A4[:, hb, :], hyT[hb], Relu, bias=bias)

            A_bufs = [
                work.tile([128, HB, B], F16, tag="A4", bufs=3, name=f"A4_{p}")
                for p in range(3)
            ]

            gen_A(0, A_bufs[0])
            for i in range(BI):
                A4 = A_bufs[i % 3]
                if i + 1 < BI:
                    gen_A(i + 1, A_bufs[(i + 1) % 3])
                for jc in range(JC):
                    pz = ps.tile(
                        [128, B], FP, tag="pz", bufs=4, name=f"pz{i}_{jc}"
                    )
                    if b2_nonzero:
                        nc.tensor.matmul(pz, ones_st, b2row, start=True, stop=False)
                    for hb in range(HB):
                        nc.tensor.matmul(
                            pz,
                            A4[:, hb, jc * 128 : (jc + 1) * 128],
                            w2T_sb[:, hb, :],
                            start=(hb == 0 and not b2_nonzero),
                            stop=(hb == HB - 1),
                        )
                    # acc[jc][:, i] = sum_k relu(pz) * w3
                    nc.vector.scalar_tensor_tensor(
                        dummy,
                        pz,
                        0.0,
                        w3bc,
                        Max,
                        Mult,
                        accum_out=acc[jc][:, i : i + 1],
                    )

            # ---------------- tail: +b3, DMA out (permuted layout) ----------
            out_q = [nc.sync, nc.scalar, nc.sync, nc.scalar]
            for jc in range(JC):
                outj = persist.tile([128, BI], FP, name=f"outj{jc}")
                nc.vector.tensor_scalar(outj, acc[jc], b3bc, 0.0, Add, Bypass)
                out_q[jc].dma_start(out_d[:, jc, :], outj)

    nc.compile()
    return nc


_BUILT: dict[str, bass.Bass] = {}


def _get_nc(key: str) -> bass.Bass:
    if key not in _BUILT:
        _BUILT[key] = build_v18(b2_nonzero=(key == "b2"))
    return _BUILT[key]


def run(inputs: dict, variant: str | None = None, trace: bool = False):
    x16 = np.asarray(inputs["x"], dtype=np.float32).astype(np.float16)
    y16 = np.asarray(inputs["y"], dtype=np.float32).astype(np.float16)
    W1 = np.asarray(inputs["W1"], dtype=np.float32).astype(np.float16)
    W2 = np.asarray(inputs["W2"], dtype=np.float32).astype(np.float16)
    b1 = np.ascontiguousarray(np.asarray(inputs["b1"], dtype=np.float32))
    b2 = np.ascontiguousarray(np.asarray(inputs["b2"], dtype=np.float32))
    W3 = np.asarray(inputs["W3"], dtype=np.float32)
    b3 = np.asarray(inputs["b3"], dtype=np.float32)

    # yT[d, r*128 + c] = y[4c + r, d]
    yT = np.ascontiguousarray(
        y16.T.reshape(D, 128, 4).transpose(0, 2, 1).reshape(D, B)
    )
    # w1T[d, hb, 0, c] = W1[4c+hb, d]; w1T[d, hb, 1, c] = W1[4c+hb, D+d]
    W1v = W1.reshape(128, 4, 2 * D)
    w1x = W1v[:, :, :D].transpose(2, 1, 0)  # [d, hb, c]
    w1y = W1v[:, :, D:].transpose(2, 1, 0)  # [d, hb, c]
    w1T = np.ascontiguousarray(np.stack((w1x, w1y), axis=2))  # [d, hb, 2, c]
    # w2T[c, hb, r*128 + p] = W2[4p + r, 4c + hb]
    w2T = np.ascontiguousarray(
        W2.reshape(128, 4, 128, 4).transpose(2, 3, 1, 0).reshape(128, HB, H)
    )
    # permuted w3 row: w3p[r*128 + c] = W3[0, 4c + r]
    w3p = np.ascontiguousarray(
        W3[0].reshape(128, 4).T.reshape(1, -1).astype(np.float32)
    )
    b3c = np.ascontiguousarray(b3.reshape(1, 1).astype(np.float32))

    nc = _get_nc("b2" if np.any(b2) else "z")
    in_maps = []
    for c in range(NCORES):
        xsT = np.ascontiguousarray(x16[c * BI : (c + 1) * BI].T)  # [D, BI]
        in_maps.append(
            {
                "xsT": xsT,
                "yT": yT,
                "w1T": w1T,
                "b1": b1,
                "w2T": w2T,
                "b2": b2,
                "w3p": w3p,
                "b3c": b3c,
            }
        )
    res = run_bass_kernel_spmd(nc, in_maps, core_ids=list(range(NCORES)), trace=trace)
    # r["out"][c, jc, i] = score[i, 4c + jc] -> transpose to [i, c, jc] and
    # flatten: column index c*4 + jc = j.
    out = np.concatenate(
        [r["out"].transpose(2, 0, 1).reshape(BI, B) for r in res.results], axis=0
    )
    return np.ascontiguousarray(out), res


def kernel(**inputs) -> np.ndarray:
    out, _ = run(inputs)
    return out


# revision 36
# speedup vs baseline: 1.0113x; 1.0004x over previous
"""ConcatCritic pair-grid MLP, v23: host-pretransposed fp16 layouts.

The host shard step prepares data layouts (pure data movement + fp16
casts): xsT = x-slab.T, yT/W1T/W2T pre-transposed and permuted, w3 row
permuted.  The device does zero layout transposes — its PE stream is
pure matmuls (11-MM HAM warmup -> mm1 -> pair-grid loop), so the clock
gate stays at 2.4 GHz throughout and the main loop starts warm.  w2T is
issued LAST on the scalar queue: DMA bandwidth is fair-shared, so this
makes the critical y/w1 tensors land ~2us earlier while w2T (shortest
post-landing dependency chain) becomes the tensor that lands last.

Index convention (from contiguous [512,n] host rows -> 128 partitions):
  h = 4c + hb  (W1/b1 rows, A4 partitions, w2T rows)
  j = 4c + jc  (y rows, A4/hy columns, pz/acc partitions)
  k = 4p + r   (W2 rows, pz columns, w3/b2 columns)
Host unshard: out_d[c, jc, i] -> score[i, 4c + jc].

Per-core (64 rows of x, everything else replicated):
  setup:  mm1 -> hxbT[h, i](+b1) fp32, hyT[hb][h, j] fp16
  per i:  ACT  A4[:, hb, :] = relu(hyT[hb] + hxbT[:, hb*64+i])   (4 ops, fp16)
          PE   pz[jc][j, k] += A4[:, hb, jc*128:+128].T @ w2T[hb]  (16 matmuls)
          DVE  acc[jc][:, i] = sum_k relu(pz[jc]) * w3bc          (4 fused ops)

The W3 reduction costs zero PE time (fused into the DVE relu). b2 is
zero in this model family; a fallback build adds an exact K=1 matmul
(ones.T @ b2perm) into each psum accumulation when b2 != 0.
"""

import os

import numpy as np

import concourse.bass as bass
import concourse.bacc as bacc
import concourse.mybir as mybir
from concourse import tile
from concourse.bass_utils import run_bass_kernel_spmd

B = 512
D = 128
H = 512
NCORES = 8
BI = B // NCORES  # 64 rows of x per core
HB = H // 128     # 4 h-blocks
JC = B // 128     # 4 j-chunks
FP = mybir.dt.float32
F16 = mybir.dt.float16

Relu = mybir.ActivationFunctionType.Relu
Identity = mybir.ActivationFunctionType.Identity
Add = mybir.AluOpType.add
Max = mybir.AluOpType.max
Mult = mybir.AluOpType.mult
Bypass = mybir.AluOpType.bypass


def build_v23(b2_nonzero: bool = False) -> bass.Bass:
    nc = bacc.Bacc(
        "TRN2",
        target_bir_lowering=False,
        debug=False,
        enable_asserts=False,
    )

    xsT_d = nc.dram_tensor("xsT", [D, BI], F16, kind="ExternalInput")
    yT_d = nc.dram_tensor("yT", [D, B], F16, kind="ExternalInput")
    w1T_d = nc.dram_tensor("w1T", [D, HB, 2, 128], F16, kind="ExternalInput")
    b1_d = nc.dram_tensor("b1", [H], FP, kind="ExternalInput")
    w2T_d = nc.dram_tensor("w2T", [128, HB, H], F16, kind="ExternalInput")
    b2_d = nc.dram_tensor("b2", [H], FP, kind="ExternalInput")
    w3p_d = nc.dram_tensor("w3p", [1, H], FP, kind="ExternalInput")
    b3c_d = nc.dram_tensor("b3c", [1, 1], FP, kind="ExternalInput")
    # out_d[c, jc, i] = score[i, 4c + jc]; host reshapes at unshard.
    out_d = nc.dram_tensor("out", [128, JC, BI], FP, kind="ExternalOutput")

    with tile.TileContext(nc) as tc:
        with (
            tc.tile_pool(name="consts", bufs=1) as consts,
            tc.tile_pool(name="persist", bufs=1) as persist,
            tc.tile_pool(name="load", bufs=1) as load,
            tc.tile_pool(name="work", bufs=3) as work,
            tc.tile_pool(name="ps", bufs=8, space="PSUM") as ps,
        ):
            # HAM warmup: N=512 matmuls on memset garbage while DMAs land —
            # keeps the PE clock gate at 2.4 GHz into mm1 and the main loop.
            warm_src = consts.tile([128, B], F16, name="warm_src")
            nc.gpsimd.memset(warm_src, 0.0)
            warm_ps = ps.tile([128, B], FP, tag="misc", bufs=1, name="warm_ps")
            for _ in range(11):
                nc.tensor.matmul(
                    warm_ps, warm_src[:, :128], warm_src, start=True, stop=True
                )

            # ------- input DMAs: contiguous fp16 pre-transposed loads -------
            xsT = load.tile([D, BI], F16, name="xsT")
            nc.sync.dma_start(xsT, xsT_d[:, :])
            w1T_sb = load.tile([D, HB, 2, 128], F16, name="w1T_sb")
            nc.sync.dma_start(w1T_sb, w1T_d[:, :, :, :])
            yT = load.tile([D, B], F16, name="yT")
            nc.sync.dma_start(yT, yT_d[:, :])
            # w2T issued LAST on scalar (its descgen delay deprioritizes
            # the big 512KB transfer so y/w1 land first; w2T is only needed
            # when the pair-grid loop starts)
            b1c = consts.tile([128, HB], FP, name="b1c")
            nc.scalar.dma_start(b1c, b1_d[:].rearrange("(p r) -> p r", p=128))
            # host-permuted w3 row + b3, broadcast on idle GpSimd
            w3prow = consts.tile([1, H], FP, name="w3prow")
            nc.scalar.dma_start(w3prow, w3p_d[:, :])
            b3c = consts.tile([1, 1], FP, name="b3c")
            nc.scalar.dma_start(b3c, b3c_d[:, :])
            w2T_sb = load.tile([128, HB, H], F16, name="w2T_sb")
            nc.scalar.dma_start(w2T_sb, w2T_d[:, :, :])
            w3bc = consts.tile([128, B], FP, name="w3bc")
            nc.gpsimd.partition_broadcast(w3bc[:, :], w3prow[:, :])
            b3bc = consts.tile([128, 1], FP, name="b3bc")
            nc.gpsimd.partition_broadcast(b3bc[:, :], b3c[:, :])
            if b2_nonzero:
                b2row = consts.tile([1, H], F16, name="b2row")
                b2row32 = consts.tile([1, H], FP, name="b2row32")
                nc.scalar.dma_start(b2row32, b2_d[None, :])
                b2p32 = consts.tile([1, H], FP, name="b2p32")
                for r in range(4):
                    nc.vector.tensor_copy(
                        b2p32[:, r * 128 : (r + 1) * 128], b2row32[:, r::4]
                    )
                nc.vector.tensor_copy(b2row, b2p32)
                ones_st = consts.tile([1, 128], F16, name="ones_st")
                nc.vector.memset(ones_st, 1.0)

            # ---------------- mm1 (no transposes needed) ----------------
            # hxbT[c, hb*BI + i] = hx[i, 4c+hb] + b1[4c+hb]   (fp32)
            hxbT = persist.tile([128, HB * BI], FP, name="hxbT")
            hyT = [persist.tile([128, B], F16, name=f"hyT{hb}") for hb in range(HB)]
            hy_ps_l = []
            for hb in range(HB):
                hx_ps = ps.tile([128, BI], FP, tag="tbank", bufs=2, name=f"hx_ps{hb}")
                nc.tensor.matmul(
                    hx_ps, w1T_sb[:, hb, 0, :], xsT, start=True, stop=True
                )
                if hb % 2 == 0:
                    nc.vector.tensor_scalar(
                        hxbT[:, hb * BI : (hb + 1) * BI],
                        hx_ps,
                        b1c[:, hb : hb + 1],
                        0.0,
                        Add,
                        Bypass,
                    )
                else:
                    nc.scalar.activation(
                        hxbT[:, hb * BI : (hb + 1) * BI],
                        hx_ps,
                        Identity,
                        bias=b1c[:, hb : hb + 1],
                    )
                hy_ps = ps.tile([128, B], FP, tag="pz", bufs=4, name=f"hy_ps{hb}")
                nc.tensor.matmul(
                    hy_ps, w1T_sb[:, hb, 1, :], yT, start=True, stop=True
                )
                nc.vector.tensor_copy(
                    hyT[hb][:, : B // 2], hy_ps[:, : B // 2]
                )
                nc.scalar.activation(
                    hyT[hb][:, B // 2 :], hy_ps[:, B // 2 :], Identity
                )
                hy_ps_l.append(hy_ps)

            # accumulator staging: acc[jc][c, i] = score[i, 4c + jc]
            acc = [persist.tile([128, BI], FP, name=f"acc{jc}") for jc in range(JC)]
            dummy = persist.tile([128, B], F16, name="stt_dummy")

            # ---------------- main loop ----------------
            def gen_A(i, A4):
                for hb in range(HB):
                    bias = hxbT[:, hb * BI + i : hb * BI + i + 1]
                    if i == 0:
                        nc.vector.tensor_scalar(
                            A4[:, hb, :], hyT[hb], bias, 0.0, Add, Max
                        )
                    else:
                        nc.scalar.activation(A4[:, hb, :], hyT[hb], Relu, bias=bias)

            A_bufs = [
                work.tile([128, HB, B], F16, tag="A4", bufs=3, name=f"A4_{p}")
                for p in range(3)
            ]

            gen_A(0, A_bufs[0])
            for i in range(BI):
                A4 = A_bufs[i % 3]
                if i + 1 < BI:
                    gen_A(i + 1, A_bufs[(i + 1) % 3])
                for jc in range(JC):
                    pz = ps.tile(
                        [128, B], FP, tag="pz", bufs=4, name=f"pz{i}_{jc}"
                    )
                    if b2_nonzero:
                        nc.tensor.matmul(pz, ones_st, b2row, start=True, stop=False)
                    for hb in range(HB):
                        nc.tensor.matmul(
                            pz,
                            A4[:, hb, jc * 128 : (jc + 1) * 128],
                            w2T_sb[:, hb, :],
                            start=(hb == 0 and not b2_nonzero),
                            stop=(hb == HB - 1),
                        )
                    # acc[jc][:, i] = sum_k relu(pz) * w3
                    nc.vector.scalar_tensor_tensor(
                        dummy,
                        pz,
                        0.0,
                        w3bc,
                        Max,
                        Mult,
                        accum_out=acc[jc][:, i : i + 1],
                    )

            # ---------------- tail: +b3, DMA out (permuted layout) ----------
            out_q = [nc.sync, nc.scalar, nc.sync, nc.scalar]
            for jc in range(JC):
                outj = persist.tile([128, BI], FP, name=f"outj{jc}")
                nc.vector.tensor_scalar(outj, acc[jc], b3bc, 0.0, Add, Bypass)
                out_q[jc].dma_start(out_d[:, jc, :], outj)

    nc.compile()
    return nc


_BUILT: dict[str, bass.Bass] = {}


def _get_nc(key: str) -> bass.Bass:
    if key not in _BUILT:
        _BUILT[key] = build_v23(b2_nonzero=(key == "b2"))
    return _BUILT[key]


def run(inputs: dict, variant: str | None = None, trace: bool = False):
    x16 = np.asarray(inputs["x"], dtype=np.float32).astype(np.float16)
    y16 = np.asarray(inputs["y"], dtype=np.float32).astype(np.float16)
    W1 = np.asarray(inputs["W1"], dtype=np.float32).astype(np.float16)
    W2 = np.asarray(inputs["W2"], dtype=np.float32).astype(np.float16)
    b1 = np.ascontiguousarray(np.asarray(inputs["b1"], dtype=np.float32))
    b2 = np.ascontiguousarray(np.asarray(inputs["b2"], dtype=np.float32))
    W3 = np.asarray(inputs["W3"], dtype=np.float32)
    b3 = np.asarray(inputs["b3"], dtype=np.float32)

    # yT[d, r*128 + c] = y[4c + r, d]
    yT = np.ascontiguousarray(
        y16.T.reshape(D, 128, 4).transpose(0, 2, 1).reshape(D, B)
    )
    # w1T[d, hb, 0, c] = W1[4c+hb, d]; w1T[d, hb, 1, c] = W1[4c+hb, D+d]
    W1v = W1.reshape(128, 4, 2 * D)
    w1x = W1v[:, :, :D].transpose(2, 1, 0)  # [d, hb, c]
    w1y = W1v[:, :, D:].transpose(2, 1, 0)  # [d, hb, c]
    w1T = np.ascontiguousarray(np.stack((w1x, w1y), axis=2))  # [d, hb, 2, c]
    # w2T[c, hb, r*128 + p] = W2[4p + r, 4c + hb]
    w2T = np.ascontiguousarray(
        W2.reshape(128, 4, 128, 4).transpose(2, 3, 1, 0).reshape(128, HB, H)
    )
    # permuted w3 row: w3p[r*128 + c] = W3[0, 4c + r]
    w3p = np.ascontiguousarray(
        W3[0].reshape(128, 4).T.reshape(1, -1).astype(np.float32)
    )
    b3c = np.ascontiguousarray(b3.reshape(1, 1).astype(np.float32))

    nc = _get_nc("b2" if np.any(b2) else "z")
    in_maps = []
    for c in range(NCORES):
        xsT = np.ascontiguousarray(x16[c * BI : (c + 1) * BI].T)  # [D, BI]
        in_maps.append(
            {
                "xsT": xsT,
                "yT": yT,
                "w1T": w1T,
                "b1": b1,
                "w2T": w2T,
                "b2": b2,
                "w3p": w3p,
                "b3c": b3c,
            }
        )
    res = run_bass_kernel_spmd(nc, in_maps, core_ids=list(range(NCORES)), trace=trace)
    # r["out"][c, jc, i] = score[i, 4c + jc] -> transpose to [i, c, jc] and
    # flatten: column index c*4 + jc = j.
    out = np.concatenate(
        [r["out"].transpose(2, 0, 1).reshape(BI, B) for r in res.results], axis=0
    )
    return np.ascontiguousarray(out), res


def kernel(**inputs) -> np.ndarray:
    out, _ = run(inputs)
    return out


# revision 37
# speedup vs baseline: 1.0158x; 1.0044x over previous
"""ConcatCritic pair-grid MLP, v23: host-pretransposed fp16 layouts.

The host shard step prepares data layouts (pure data movement + fp16
casts): xsT = x-slab.T, yT/W1T/W2T pre-transposed and permuted, w3 row
permuted.  The device does zero layout transposes — its PE stream is
pure matmuls (11-MM HAM warmup -> mm1 -> pair-grid loop), so the clock
gate stays at 2.4 GHz throughout and the main loop starts warm.  w2T is
issued LAST on the scalar queue: DMA bandwidth is fair-shared, so this
makes the critical y/w1 tensors land ~2us earlier while w2T (shortest
post-landing dependency chain) becomes the tensor that lands last.

Index convention (from contiguous [512,n] host rows -> 128 partitions):
  h = 4c + hb  (W1/b1 rows, A4 partitions, w2T rows)
  j = 4c + jc  (y rows, A4/hy columns, pz/acc partitions)
  k = 4p + r   (W2 rows, pz columns, w3/b2 columns)
Host unshard: out_d[c, jc, i] -> score[i, 4c + jc].

Per-core (64 rows of x, everything else replicated):
  setup:  mm1 -> hxbT[h, i](+b1) fp32, hyT[hb][h, j] fp16
  per i:  ACT  A4[:, hb, :] = relu(hyT[hb] + hxbT[:, hb*64+i])   (4 ops, fp16)
          PE   pz[jc][j, k] += A4[:, hb, jc*128:+128].T @ w2T[hb]  (16 matmuls)
          DVE  acc[jc][:, i] = sum_k relu(pz[jc]) * w3bc          (4 fused ops)

The W3 reduction costs zero PE time (fused into the DVE relu). b2 is
zero in this model family; a fallback build adds an exact K=1 matmul
(ones.T @ b2perm) into each psum accumulation when b2 != 0.
"""

import os

import numpy as np

import concourse.bass as bass
import concourse.bacc as bacc
import concourse.mybir as mybir
from concourse import tile
from concourse.bass_utils import run_bass_kernel_spmd

B = 512
D = 128
H = 512
NCORES = 8
BI = B // NCORES  # 64 rows of x per core
HB = H // 128     # 4 h-blocks
JC = B // 128     # 4 j-chunks
FP = mybir.dt.float32
F16 = mybir.dt.float16

Relu = mybir.ActivationFunctionType.Relu
Identity = mybir.ActivationFunctionType.Identity
Add = mybir.AluOpType.add
Max = mybir.AluOpType.max
Mult = mybir.AluOpType.mult
Bypass = mybir.AluOpType.bypass


def build_v23(b2_nonzero: bool = False, b3_nonzero: bool = False) -> bass.Bass:
    nc = bacc.Bacc(
        "TRN2",
        target_bir_lowering=False,
        debug=False,
        enable_asserts=False,
    )

    xsT_d = nc.dram_tensor("xsT", [D, BI], F16, kind="ExternalInput")
    yT_d = nc.dram_tensor("yT", [D, B], F16, kind="ExternalInput")
    w1T_d = nc.dram_tensor("w1T", [D, HB, 2, 128], F16, kind="ExternalInput")
    b1_d = nc.dram_tensor("b1", [H], FP, kind="ExternalInput")
    w2T_d = nc.dram_tensor("w2T", [128, HB, H], F16, kind="ExternalInput")
    b2_d = nc.dram_tensor("b2", [H], FP, kind="ExternalInput")
    w3p_d = nc.dram_tensor("w3p", [1, H], FP, kind="ExternalInput")
    b3c_d = nc.dram_tensor("b3c", [1, 1], FP, kind="ExternalInput")
    # out_d[c, jc, i] = score[i, 4c + jc]; host reshapes at unshard.
    out_d = nc.dram_tensor("out", [128, JC, BI], FP, kind="ExternalOutput")

    with tile.TileContext(nc) as tc:
        with (
            tc.tile_pool(name="consts", bufs=1) as consts,
            tc.tile_pool(name="persist", bufs=1) as persist,
            tc.tile_pool(name="load", bufs=1) as load,
            tc.tile_pool(name="work", bufs=3) as work,
            tc.tile_pool(name="ps", bufs=8, space="PSUM") as ps,
        ):
            # HAM warmup: N=512 matmuls on memset garbage while DMAs land —
            # keeps the PE clock gate at 2.4 GHz into mm1 and the main loop.
            warm_src = consts.tile([128, B], F16, name="warm_src")
            nc.gpsimd.memset(warm_src, 0.0)
            warm_ps = ps.tile([128, B], FP, tag="misc", bufs=1, name="warm_ps")
            for _ in range(11):
                nc.tensor.matmul(
                    warm_ps, warm_src[:, :128], warm_src, start=True, stop=True
                )

            # ------- input DMAs: contiguous fp16 pre-transposed loads -------
            xsT = load.tile([D, BI], F16, name="xsT")
            nc.sync.dma_start(xsT, xsT_d[:, :])
            w1T_sb = load.tile([D, HB, 2, 128], F16, name="w1T_sb")
            nc.sync.dma_start(w1T_sb, w1T_d[:, :, :, :])
            yT = load.tile([D, B], F16, name="yT")
            nc.sync.dma_start(yT, yT_d[:, :])
            # w2T issued LAST on scalar (its descgen delay deprioritizes
            # the big 512KB transfer so y/w1 land first; w2T is only needed
            # when the pair-grid loop starts)
            b1c = consts.tile([128, HB], FP, name="b1c")
            nc.scalar.dma_start(b1c, b1_d[:].rearrange("(p r) -> p r", p=128))
            # host-permuted w3 row + b3, broadcast on idle GpSimd
            w3prow = consts.tile([1, H], FP, name="w3prow")
            nc.scalar.dma_start(w3prow, w3p_d[:, :])
            b3c = consts.tile([1, 1], FP, name="b3c")
            nc.scalar.dma_start(b3c, b3c_d[:, :])
            w2T_sb = load.tile([128, HB, H], F16, name="w2T_sb")
            nc.scalar.dma_start(w2T_sb, w2T_d[:, :, :])
            w3bc = consts.tile([128, B], FP, name="w3bc")
            nc.gpsimd.partition_broadcast(w3bc[:, :], w3prow[:, :])
            if b3_nonzero:
                b3bc = consts.tile([128, 1], FP, name="b3bc")
                nc.gpsimd.partition_broadcast(b3bc[:, :], b3c[:, :])
            if b2_nonzero:
                b2row = consts.tile([1, H], F16, name="b2row")
                b2row32 = consts.tile([1, H], FP, name="b2row32")
                nc.scalar.dma_start(b2row32, b2_d[None, :])
                b2p32 = consts.tile([1, H], FP, name="b2p32")
                for r in range(4):
                    nc.vector.tensor_copy(
                        b2p32[:, r * 128 : (r + 1) * 128], b2row32[:, r::4]
                    )
                nc.vector.tensor_copy(b2row, b2p32)
                ones_st = consts.tile([1, 128], F16, name="ones_st")
                nc.vector.memset(ones_st, 1.0)

            # ---------------- mm1 (no transposes needed) ----------------
            # hxbT[c, hb*BI + i] = hx[i, 4c+hb] + b1[4c+hb]   (fp32)
            hxbT = persist.tile([128, HB * BI], FP, name="hxbT")
            hyT = [persist.tile([128, B], F16, name=f"hyT{hb}") for hb in range(HB)]
            hy_ps_l = []
            for hb in range(HB):
                hx_ps = ps.tile([128, BI], FP, tag="tbank", bufs=2, name=f"hx_ps{hb}")
                nc.tensor.matmul(
                    hx_ps, w1T_sb[:, hb, 0, :], xsT, start=True, stop=True
                )
                if hb % 2 == 0:
                    nc.vector.tensor_scalar(
                        hxbT[:, hb * BI : (hb + 1) * BI],
                        hx_ps,
                        b1c[:, hb : hb + 1],
                        0.0,
                        Add,
                        Bypass,
                    )
                else:
                    nc.scalar.activation(
                        hxbT[:, hb * BI : (hb + 1) * BI],
                        hx_ps,
                        Identity,
                        bias=b1c[:, hb : hb + 1],
                    )
                hy_ps = ps.tile([128, B], FP, tag="pz", bufs=4, name=f"hy_ps{hb}")
                nc.tensor.matmul(
                    hy_ps, w1T_sb[:, hb, 1, :], yT, start=True, stop=True
                )
                nc.vector.tensor_copy(
                    hyT[hb][:, : B // 2], hy_ps[:, : B // 2]
                )
                nc.scalar.activation(
                    hyT[hb][:, B // 2 :], hy_ps[:, B // 2 :], Identity
                )
                hy_ps_l.append(hy_ps)

            # accumulator staging: acc[jc][c, i] = score[i, 4c + jc]
            acc = [persist.tile([128, BI], FP, name=f"acc{jc}") for jc in range(JC)]
            dummy = persist.tile([128, B], F16, name="stt_dummy")

            # ---------------- main loop ----------------
            def gen_A(i, A4):
                for hb in range(HB):
                    bias = hxbT[:, hb * BI + i : hb * BI + i + 1]
                    if i == 0:
                        nc.vector.tensor_scalar(
                            A4[:, hb, :], hyT[hb], bias, 0.0, Add, Max
                        )
                    else:
                        nc.scalar.activation(A4[:, hb, :], hyT[hb], Relu, bias=bias)

            A_bufs = [
                work.tile([128, HB, B], F16, tag="A4", bufs=3, name=f"A4_{p}")
                for p in range(3)
            ]

            gen_A(0, A_bufs[0])
            for i in range(BI):
                A4 = A_bufs[i % 3]
                if i + 1 < BI:
                    gen_A(i + 1, A_bufs[(i + 1) % 3])
                for jc in range(JC):
                    pz = ps.tile(
                        [128, B], FP, tag="pz", bufs=4, name=f"pz{i}_{jc}"
                    )
                    if b2_nonzero:
                        nc.tensor.matmul(pz, ones_st, b2row, start=True, stop=False)
                    for hb in range(HB):
                        nc.tensor.matmul(
                            pz,
                            A4[:, hb, jc * 128 : (jc + 1) * 128],
                            w2T_sb[:, hb, :],
                            start=(hb == 0 and not b2_nonzero),
                            stop=(hb == HB - 1),
                        )
                    # acc[jc][:, i] = sum_k relu(pz) * w3
                    nc.vector.scalar_tensor_tensor(
                        dummy,
                        pz,
                        0.0,
                        w3bc,
                        Max,
                        Mult,
                        accum_out=acc[jc][:, i : i + 1],
                    )

            # ---------------- tail: +b3 (if any), DMA out ----------
            out_q = [nc.sync, nc.scalar, nc.sync, nc.scalar]
            for jc in range(JC):
                if b3_nonzero:
                    outj = persist.tile([128, BI], FP, name=f"outj{jc}")
                    nc.vector.tensor_scalar(outj, acc[jc], b3bc, 0.0, Add, Bypass)
                    out_q[jc].dma_start(out_d[:, jc, :], outj)
                else:
                    out_q[jc].dma_start(out_d[:, jc, :], acc[jc])

    nc.compile()
    return nc


_BUILT: dict[str, bass.Bass] = {}


def _get_nc(key: str) -> bass.Bass:
    if key not in _BUILT:
        _BUILT[key] = build_v23(
            b2_nonzero="b2" in key, b3_nonzero="b3" in key
        )
    return _BUILT[key]


def run(inputs: dict, variant: str | None = None, trace: bool = False):
    x16 = np.asarray(inputs["x"], dtype=np.float32).astype(np.float16)
    y16 = np.asarray(inputs["y"], dtype=np.float32).astype(np.float16)
    W1 = np.asarray(inputs["W1"], dtype=np.float32).astype(np.float16)
    W2 = np.asarray(inputs["W2"], dtype=np.float32).astype(np.float16)
    b1 = np.ascontiguousarray(np.asarray(inputs["b1"], dtype=np.float32))
    b2 = np.ascontiguousarray(np.asarray(inputs["b2"], dtype=np.float32))
    W3 = np.asarray(inputs["W3"], dtype=np.float32)
    b3 = np.asarray(inputs["b3"], dtype=np.float32)

    # yT[d, r*128 + c] = y[4c + r, d]
    yT = np.ascontiguousarray(
        y16.T.reshape(D, 128, 4).transpose(0, 2, 1).reshape(D, B)
    )
    # w1T[d, hb, 0, c] = W1[4c+hb, d]; w1T[d, hb, 1, c] = W1[4c+hb, D+d]
    W1v = W1.reshape(128, 4, 2 * D)
    w1x = W1v[:, :, :D].transpose(2, 1, 0)  # [d, hb, c]
    w1y = W1v[:, :, D:].transpose(2, 1, 0)  # [d, hb, c]
    w1T = np.ascontiguousarray(np.stack((w1x, w1y), axis=2))  # [d, hb, 2, c]
    # w2T[c, hb, r*128 + p] = W2[4p + r, 4c + hb]
    w2T = np.ascontiguousarray(
        W2.reshape(128, 4, 128, 4).transpose(2, 3, 1, 0).reshape(128, HB, H)
    )
    # permuted w3 row: w3p[r*128 + c] = W3[0, 4c + r]
    w3p = np.ascontiguousarray(
        W3[0].reshape(128, 4).T.reshape(1, -1).astype(np.float32)
    )
    b3c = np.ascontiguousarray(b3.reshape(1, 1).astype(np.float32))

    key = ("b2" if np.any(b2) else "") + ("b3" if np.any(b3) else "")
    nc = _get_nc(key or "z")
    in_maps = []
    for c in range(NCORES):
        xsT = np.ascontiguousarray(x16[c * BI : (c + 1) * BI].T)  # [D, BI]
        in_maps.append(
            {
                "xsT": xsT,
                "yT": yT,
                "w1T": w1T,
                "b1": b1,
                "w2T": w2T,
                "b2": b2,
                "w3p": w3p,
                "b3c": b3c,
            }
        )
    res = run_bass_kernel_spmd(nc, in_maps, core_ids=list(range(NCORES)), trace=trace)
    # r["out"][c, jc, i] = score[i, 4c + jc] -> transpose to [i, c, jc] and
    # flatten: column index c*4 + jc = j.
    out = np.concatenate(
        [r["out"].transpose(2, 0, 1).reshape(BI, B) for r in res.results], axis=0
    )
    return np.ascontiguousarray(out), res


def kernel(**inputs) -> np.ndarray:
    out, _ = run(inputs)
    return out
